# revision 34
# baseline (speedup 1.0000x reference)
"""MoE layer (top-2 of 8 experts, D=1024, H=2048) on 8 trn2 NeuronCores.

Strategy (expert-parallel, per the sharding hint):
  - Router (tiny: [16384,1024]@[1024,8]) runs on host; its output IS the
    sharding decision ("all-to-all tokens by expert assignment").
  - Core e receives the tokens routed to expert e (padded to a uniform
    capacity C, transposed to [D, C] bf16) plus expert e's weights, and
    computes yT = (relu(w1.T @ x + b1)).T-chain fully on-device:
        stage 1: hT[h, c] = relu(sum_d w1[d,h] * xT[d,c] + b1[h])
        stage 2: yT[d, c] = sum_h w2[h,d] * hT[h,c]
    bf16 matmuls, f32 PSUM accumulation, f32 output.
  - Host scatter-adds gate * (y + b2) into the output.
"""

import numpy as np
import ml_dtypes

import concourse.bacc as bacc
import concourse.mybir as mybir
import concourse.tile as tile
from concourse.tile_rust import add_dep_helper
from concourse import bass_utils

B, S, D, E, TOPK = 4, 4096, 1024, 8, 2
H = 2 * D
P = 128
KD = D // P    # 8 k-tiles over D
MH = H // P    # 16 h-tiles over H
ND = D // P    # 8 d-tiles over D
FD = 512       # moving free-dim per matmul / column block
N_CORES = 8

BF16 = mybir.dt.bfloat16
F32 = mybir.dt.float32

_cache = {}           # capacity C -> compiled Bacc
last_run_results = None  # BassKernelResults of the most recent device run


def _build(C, with_b1=True):
    """Build + compile the per-core FFN program for token capacity C.

    Weight dram layouts are tile-major (host pre-transposes):
      w1m[m, k, p, j] = w1[k*128+p, m*128+j]   (m-major: stage-1 weight
        column-tile m is a contiguous 32KB chunk per k -> the first
        matmul only needs w1m[0] + the first x block, so compute starts
        ~2us into the kernel instead of after the full 8MB weight load)
      w2d[d, m, p, j] = w2[m*128+p, d*128+j]   (d-major, same idea)
    """
    assert C % P == 0
    nc = bacc.Bacc("TRN2", target_bir_lowering=False, debug=False)
    xT = nc.dram_tensor("xT", [D, C], BF16, kind="ExternalInput").ap()
    w1m = nc.dram_tensor("w1m", [MH, P, KD, P], BF16, kind="ExternalInput").ap()
    w2d = nc.dram_tensor("w2d", [ND, P, MH, P], BF16, kind="ExternalInput").ap()
    b1t = (
        nc.dram_tensor("b1t", [P, MH], F32, kind="ExternalInput").ap()
        if with_b1 else None
    )
    yT = nc.dram_tensor("yT", [D, C], F32, kind="ExternalOutput").ap()

    blocks = []
    c0 = 0
    while c0 < C:
        fd = min(FD, C - c0)
        blocks.append((c0, fd))
        c0 += fd

    xT_r = xT.rearrange("(k p) c -> p k c", p=P)      # [P, KD, C]
    yT_r = yT.rearrange("(d p) c -> p d c", p=P)      # [P, ND, C]
    w1m_r = w1m.rearrange("m p k j -> p m (k j)")     # [P, MH, KD*P]
    w2d_r = w2d.rearrange("d p m j -> p d (m j)")     # [P, ND, MH*P]

    with tile.TileContext(nc) as tc:
        with (
            tc.tile_pool(name="wpool", bufs=1) as wpool,
            tc.tile_pool(name="xpool", bufs=3) as xpool,
            tc.tile_pool(name="hpool", bufs=3) as hpool,
            tc.tile_pool(name="ypool", bufs=2) as ypool,
            tc.tile_pool(name="ps1", bufs=4, space="PSUM") as ps1,
            tc.tile_pool(name="ps2", bufs=4, space="PSUM") as ps2,
        ):
            # First x block is the very first DMA on the sync ring: the
            # first matmul depends only on it and the first w1 chunk.
            # block 0's x arrives as per-k chunks: the first accumulation
            # group starts on chunk 0 while later chunks are in flight
            xb0 = xpool.tile([P, KD, FD], BF16)
            c00, fd0 = blocks[0]
            for k in range(KD):
                nc.sync.dma_start(
                    xb0[:, k, :fd0], xT_r[:, k, c00:c00 + fd0]
                )
            if with_b1:
                b1_sb = wpool.tile([P, MH], F32)
                nc.gpsimd.dma_start(b1_sb[:], b1t[:, :])

            # w1 m=0..3 on the scalar HWDGE ring (lands early; only 4
            # triggers so the scalar engine is free for relu right after);
            # the bulk of the weights streams on the gpsimd SWDGE path as
            # a few large DMAs so no engine's trigger queue backs up.
            # A tiny SBUF->SBUF gpsimd DMA that reads xb0 gates the bulk
            # stream so it can't starve the first x block of HBM bandwidth.
            w1_sb = wpool.tile([P, MH, KD * P], BF16)
            for m in range(4):
                nc.scalar.dma_start(w1_sb[:, m, :], w1m_r[:, m, :])
            # DVE copy reading xb0 = completes only once xb0 is in SBUF;
            # every bulk weight DMA trigger gets a dep edge on it
            gate_sb = wpool.tile([P, KD], BF16)
            gate_copy = nc.vector.tensor_copy(gate_sb[:1, :], xb0[:1, :, 0])
            w2_sb = wpool.tile([P, ND, MH * P], BF16)
            bulk = []
            for m0 in range(4, MH, 4):
                bulk.append(nc.gpsimd.dma_start(
                    w1_sb[:, m0:m0 + 4, :], w1m_r[:, m0:m0 + 4, :]
                ))
            for d0 in range(0, ND, 4):
                bulk.append(nc.gpsimd.dma_start(
                    w2_sb[:, d0:d0 + 4, :], w2d_r[:, d0:d0 + 4, :]
                ))
            for w in bulk[1:]:  # m4-7 streams ungated (needed ~18us)
                add_dep_helper(
                    w.ins, gate_copy.ins,
                    reason="bulk weights wait for x block 0 to land",
                )

            xbs = {}

            def stage1(blki):
                c0, fd = blocks[blki]
                if blki == 0:
                    xb = xb0
                else:
                    xb = xpool.tile([P, KD, FD], BF16)
                    nc.sync.dma_start(xb[:, :, :fd], xT_r[:, :, c0:c0 + fd])
                hT = hpool.tile([P, MH, FD], BF16)
                xbs[blki] = (xb, hT)
                for m in range(MH):
                    ps = ps1.tile([P, FD], F32)
                    for k in range(KD):
                        nc.tensor.matmul(
                            ps[:, :fd],
                            w1_sb[:, m, k * P:(k + 1) * P],
                            xb[:, k, :fd],
                            start=(k == 0),
                            stop=(k == KD - 1),
                        )
                    nc.scalar.activation(
                        hT[:, m, :fd],
                        ps[:, :fd],
                        mybir.ActivationFunctionType.Relu,
                        bias=b1_sb[:, m:m + 1] if with_b1 else 0.0,
                    )

            def stage2(blki):
                c0, fd = blocks[blki]
                last = blki == len(blocks) - 1
                _, hT = xbs.pop(blki)
                yb = ypool.tile([P, ND, FD], F32)
                for d in range(ND):
                    ps = ps2.tile([P, FD], F32)
                    for m in range(MH):
                        nc.tensor.matmul(
                            ps[:, :fd],
                            w2_sb[:, d, m * P:(m + 1) * P],
                            hT[:, m, :fd],
                            start=(m == 0),
                            stop=(m == MH - 1),
                        )
                    nc.vector.tensor_copy(yb[:, d, :fd], ps[:, :fd])
                    if last:  # stream the tail out so the final DMA is tiny
                        nc.sync.dma_start(
                            yT_r[:, d, c0:c0 + fd], yb[:, d, :fd]
                        )
                if not last:
                    nc.sync.dma_start(yT_r[:, :, c0:c0 + fd], yb[:, :, :fd])

            # software-pipelined: stage 1 of block b+1 runs (on PE) between
            # stage 1 and stage 2 of block b, hiding the relu-eviction tail
            stage1(0)
            for b in range(len(blocks)):
                if b + 1 < len(blocks):
                    stage1(b + 1)
                stage2(b)

    nc.compile()
    return nc


def _route(x_flat, router_w, router_b):
    """Replicates the reference router in numpy float32."""
    logits = x_flat @ router_w + router_b            # [N, E]
    m = logits.max(axis=-1, keepdims=True)
    p = np.exp(logits - m, dtype=np.float32)
    p /= p.sum(axis=-1, keepdims=True)
    # top-k, ties -> lower index first (matches jax.lax.top_k)
    top_i = np.argsort(-p, axis=-1, kind="stable")[:, :TOPK]
    top_p = np.take_along_axis(p, top_i, axis=-1)
    top_p = top_p / top_p.sum(axis=-1, keepdims=True)
    return top_p, top_i


def kernel(x, router_w, router_b, w1, b1, w2, b2, _trace=False):
    global last_run_results
    x = np.asarray(x, dtype=np.float32)
    router_w = np.asarray(router_w, dtype=np.float32)
    router_b = np.asarray(router_b, dtype=np.float32)
    w1 = np.asarray(w1, dtype=np.float32)
    b1 = np.asarray(b1, dtype=np.float32)
    w2 = np.asarray(w2, dtype=np.float32)
    b2 = np.asarray(b2, dtype=np.float32)

    N = B * S
    x_flat = x.reshape(N, D)
    top_p, top_i = _route(x_flat, router_w, router_b)

    # Tokens per expert (the "all-to-all by expert assignment")
    idx = [np.nonzero((top_i == e).any(axis=-1))[0] for e in range(E)]
    gates = [
        (top_p[idx[e]] * (top_i[idx[e]] == e)).sum(axis=-1) for e in range(E)
    ]
    counts = np.array([len(i) for i in idx])
    # Device capacity: the perfect-balance point (N*K/8, multiple of 512).
    # The few overflow tokens of hotter-than-average experts are handled
    # on the host during the scatter-add (a data-parallel remainder).
    C = max(FD, int(-(-(N * TOPK // N_CORES) // FD) * FD))

    with_b1 = bool(np.any(b1))
    key = (C, with_b1)
    if key not in _cache:
        _cache[key] = _build(C, with_b1=with_b1)
    nc = _cache[key]

    in_maps = []
    for e in range(E):
        n_e = min(int(counts[e]), C)
        xTe = np.zeros((D, C), dtype=ml_dtypes.bfloat16)
        xTe[:, :n_e] = x_flat[idx[e][:n_e]].T
        w1m = np.ascontiguousarray(
            w1[e].reshape(KD, P, MH, P).transpose(2, 1, 0, 3)
        ).astype(ml_dtypes.bfloat16)
        w2d = np.ascontiguousarray(
            w2[e].reshape(MH, P, ND, P).transpose(2, 1, 0, 3)
        ).astype(ml_dtypes.bfloat16)
        im = {
            "xT": xTe,
            "w1m": w1m,
            "w2d": w2d,
        }
        if with_b1:
            im["b1t"] = np.ascontiguousarray(b1[e].reshape(MH, P).T)
        in_maps.append(im)

    res = None
    for attempt in range(3):
        try:
            res = bass_utils.run_bass_kernel_spmd(
                nc, in_maps, list(range(N_CORES)), trace=_trace
            )
            break
        except Exception:
            if attempt == 2:
                raise
    last_run_results = res

    out_flat = np.zeros((N, D), dtype=np.float32)
    for e in range(E):
        n_e = min(int(counts[e]), C)
        y_e = res.results[e]["yT"][:, :n_e].T   # [n_e, D] f32
        out_flat[idx[e][:n_e]] += gates[e][:n_e, None] * (y_e + b2[e])
        if counts[e] > C:  # host handles the overflow tokens
            hi = idx[e][C:]
            h = np.maximum(x_flat[hi] @ w1[e] + b1[e], 0.0)
            y = h @ w2[e] + b2[e]
            out_flat[hi] += gates[e][C:, None] * y
    return out_flat.reshape(B, S, D)


# revision 36
# speedup vs baseline: 1.1975x; 1.1975x over previous
"""MoE layer (top-2 of 8 experts, D=1024, H=2048) on 8 trn2 NeuronCores.

Strategy (expert-parallel, per the sharding hint):
  - Router (tiny: [16384,1024]@[1024,8]) runs on host; its output IS the
    sharding decision ("all-to-all tokens by expert assignment").
  - Core e receives the tokens routed to expert e (padded to a uniform
    capacity C, transposed to [D, C] bf16) plus expert e's weights, and
    computes yT = (relu(w1.T @ x + b1)).T-chain fully on-device:
        stage 1: hT[h, c] = relu(sum_d w1[d,h] * xT[d,c] + b1[h])
        stage 2: yT[d, c] = sum_h w2[h,d] * hT[h,c]
    bf16 matmuls, f32 PSUM accumulation, f32 output.
  - Host scatter-adds gate * (y + b2) into the output.
"""

import numpy as np
import ml_dtypes

import concourse.bacc as bacc
import concourse.mybir as mybir
import concourse.tile as tile
from concourse.tile_rust import add_dep_helper
from concourse import bass_utils

B, S, D, E, TOPK = 4, 4096, 1024, 8, 2
H = 2 * D
P = 128
KD = D // P    # 8 k-tiles over D
MH = H // P    # 16 h-tiles over H
ND = D // P    # 8 d-tiles over D
FD = 512       # moving free-dim per matmul / column block
N_CORES = 8

BF16 = mybir.dt.bfloat16
F32 = mybir.dt.float32

_cache = {}           # capacity C -> compiled Bacc
last_run_results = None  # BassKernelResults of the most recent device run


def _build(C, with_b1=True):
    """Build + compile the per-core FFN program for token capacity C.

    Weight dram layouts are tile-major (host pre-transposes):
      w1m[m, k, p, j] = w1[k*128+p, m*128+j]   (m-major: stage-1 weight
        column-tile m is a contiguous 32KB chunk per k -> the first
        matmul only needs w1m[0] + the first x block, so compute starts
        ~2us into the kernel instead of after the full 8MB weight load)
      w2d[d, m, p, j] = w2[m*128+p, d*128+j]   (d-major, same idea)
    """
    assert C % P == 0
    nc = bacc.Bacc("TRN2", target_bir_lowering=False, debug=False)
    xT = nc.dram_tensor("xT", [D, C], BF16, kind="ExternalInput").ap()
    w1m = nc.dram_tensor("w1m", [MH, P, KD, P], BF16, kind="ExternalInput").ap()
    w2d = nc.dram_tensor("w2d", [ND, P, MH, P], BF16, kind="ExternalInput").ap()
    b1t = (
        nc.dram_tensor("b1t", [P, MH], F32, kind="ExternalInput").ap()
        if with_b1 else None
    )
    yT = nc.dram_tensor("yT", [D, C], F32, kind="ExternalOutput").ap()

    blocks = []
    c0 = 0
    while c0 < C:
        fd = min(FD, C - c0)
        blocks.append((c0, fd))
        c0 += fd

    xT_r = xT.rearrange("(k p) c -> p k c", p=P)      # [P, KD, C]
    yT_r = yT.rearrange("(d p) c -> p d c", p=P)      # [P, ND, C]
    w1m_r = w1m.rearrange("m p k j -> p m (k j)")     # [P, MH, KD*P]
    w2d_r = w2d.rearrange("d p m j -> p d (m j)")     # [P, ND, MH*P]

    with tile.TileContext(nc) as tc:
        with (
            tc.tile_pool(name="wpool", bufs=1) as wpool,
            tc.tile_pool(name="xpool", bufs=3) as xpool,
            tc.tile_pool(name="hpool", bufs=3) as hpool,
            tc.tile_pool(name="ypool", bufs=2) as ypool,
            tc.tile_pool(name="ps1", bufs=4, space="PSUM") as ps1,
            tc.tile_pool(name="ps2", bufs=4, space="PSUM") as ps2,
        ):
            # First x block is the very first DMA on the sync ring: the
            # first matmul depends only on it and the first w1 chunk.
            # block 0's x arrives as per-k chunks: the first accumulation
            # group starts on chunk 0 while later chunks are in flight
            xb0 = xpool.tile([P, KD, FD], BF16)
            c00, fd0 = blocks[0]
            for k in range(KD):
                nc.sync.dma_start(
                    xb0[:, k, :fd0], xT_r[:, k, c00:c00 + fd0]
                )
            if with_b1:
                b1_sb = wpool.tile([P, MH], F32)
                nc.gpsimd.dma_start(b1_sb[:], b1t[:, :])

            # w1 m=0..3 on the scalar HWDGE ring (lands early; only 4
            # triggers so the scalar engine is free for relu right after);
            # the bulk of the weights streams on the gpsimd SWDGE path as
            # a few large DMAs so no engine's trigger queue backs up.
            # A tiny SBUF->SBUF gpsimd DMA that reads xb0 gates the bulk
            # stream so it can't starve the first x block of HBM bandwidth.
            w1_sb = wpool.tile([P, MH, KD * P], BF16)
            for m in range(4):
                nc.scalar.dma_start(w1_sb[:, m, :], w1m_r[:, m, :])
            # DVE copy reading xb0 = completes only once xb0 is in SBUF;
            # every bulk weight DMA trigger gets a dep edge on it
            gate_sb = wpool.tile([P, 8], BF16)
            gate_copy = nc.vector.tensor_copy(gate_sb[:1, :8], xb0[:1, 0, :8])
            w2_sb = wpool.tile([P, ND, MH * P], BF16)
            bulk = []
            for m0 in range(4, MH, 4):
                bulk.append(nc.gpsimd.dma_start(
                    w1_sb[:, m0:m0 + 4, :], w1m_r[:, m0:m0 + 4, :]
                ))
            for d0 in range(0, ND, 4):
                bulk.append(nc.gpsimd.dma_start(
                    w2_sb[:, d0:d0 + 4, :], w2d_r[:, d0:d0 + 4, :]
                ))
            for w in bulk:
                add_dep_helper(
                    w.ins, gate_copy.ins,
                    reason="bulk weights wait for x chunk 0 to land",
                )

            xbs = {}

            def stage1(blki):
                c0, fd = blocks[blki]
                if blki == 0:
                    xb = xb0
                else:
                    xb = xpool.tile([P, KD, FD], BF16)
                    nc.sync.dma_start(xb[:, :, :fd], xT_r[:, :, c0:c0 + fd])
                hT = hpool.tile([P, MH, FD], BF16)
                xbs[blki] = (xb, hT)
                for m in range(MH):
                    ps = ps1.tile([P, FD], F32)
                    for k in range(KD):
                        nc.tensor.matmul(
                            ps[:, :fd],
                            w1_sb[:, m, k * P:(k + 1) * P],
                            xb[:, k, :fd],
                            start=(k == 0),
                            stop=(k == KD - 1),
                        )
                    nc.scalar.activation(
                        hT[:, m, :fd],
                        ps[:, :fd],
                        mybir.ActivationFunctionType.Relu,
                        bias=b1_sb[:, m:m + 1] if with_b1 else 0.0,
                    )

            def stage2(blki):
                c0, fd = blocks[blki]
                last = blki == len(blocks) - 1
                _, hT = xbs.pop(blki)
                yb = ypool.tile([P, ND, FD], F32)
                for d in range(ND):
                    ps = ps2.tile([P, FD], F32)
                    for m in range(MH):
                        nc.tensor.matmul(
                            ps[:, :fd],
                            w2_sb[:, d, m * P:(m + 1) * P],
                            hT[:, m, :fd],
                            start=(m == 0),
                            stop=(m == MH - 1),
                        )
                    nc.vector.tensor_copy(yb[:, d, :fd], ps[:, :fd])
                    if last:  # stream the tail out so the final DMA is tiny
                        nc.sync.dma_start(
                            yT_r[:, d, c0:c0 + fd], yb[:, d, :fd]
                        )
                if not last:
                    nc.sync.dma_start(yT_r[:, :, c0:c0 + fd], yb[:, :, :fd])

            # software-pipelined: stage 1 of block b+1 runs (on PE) between
            # stage 1 and stage 2 of block b, hiding the relu-eviction tail
            stage1(0)
            for b in range(len(blocks)):
                if b + 1 < len(blocks):
                    stage1(b + 1)
                stage2(b)

    nc.compile()
    return nc


def _route(x_flat, router_w, router_b):
    """Replicates the reference router in numpy float32."""
    logits = x_flat @ router_w + router_b            # [N, E]
    m = logits.max(axis=-1, keepdims=True)
    p = np.exp(logits - m, dtype=np.float32)
    p /= p.sum(axis=-1, keepdims=True)
    # top-k, ties -> lower index first (matches jax.lax.top_k)
    top_i = np.argsort(-p, axis=-1, kind="stable")[:, :TOPK]
    top_p = np.take_along_axis(p, top_i, axis=-1)
    top_p = top_p / top_p.sum(axis=-1, keepdims=True)
    return top_p, top_i


def kernel(x, router_w, router_b, w1, b1, w2, b2, _trace=False):
    global last_run_results
    x = np.asarray(x, dtype=np.float32)
    router_w = np.asarray(router_w, dtype=np.float32)
    router_b = np.asarray(router_b, dtype=np.float32)
    w1 = np.asarray(w1, dtype=np.float32)
    b1 = np.asarray(b1, dtype=np.float32)
    w2 = np.asarray(w2, dtype=np.float32)
    b2 = np.asarray(b2, dtype=np.float32)

    N = B * S
    x_flat = x.reshape(N, D)
    top_p, top_i = _route(x_flat, router_w, router_b)

    # Tokens per expert (the "all-to-all by expert assignment")
    idx = [np.nonzero((top_i == e).any(axis=-1))[0] for e in range(E)]
    gates = [
        (top_p[idx[e]] * (top_i[idx[e]] == e)).sum(axis=-1) for e in range(E)
    ]
    counts = np.array([len(i) for i in idx])
    # Device capacity: the perfect-balance point (N*K/8, multiple of 512).
    # The few overflow tokens of hotter-than-average experts are handled
    # on the host during the scatter-add (a data-parallel remainder).
    C = max(FD, int(-(-(N * TOPK // N_CORES) // FD) * FD))

    with_b1 = bool(np.any(b1))
    key = (C, with_b1)
    if key not in _cache:
        _cache[key] = _build(C, with_b1=with_b1)
    nc = _cache[key]

    in_maps = []
    for e in range(E):
        n_e = min(int(counts[e]), C)
        xTe = np.zeros((D, C), dtype=ml_dtypes.bfloat16)
        xTe[:, :n_e] = x_flat[idx[e][:n_e]].T
        w1m = np.ascontiguousarray(
            w1[e].reshape(KD, P, MH, P).transpose(2, 1, 0, 3)
        ).astype(ml_dtypes.bfloat16)
        w2d = np.ascontiguousarray(
            w2[e].reshape(MH, P, ND, P).transpose(2, 1, 0, 3)
        ).astype(ml_dtypes.bfloat16)
        im = {
            "xT": xTe,
            "w1m": w1m,
            "w2d": w2d,
        }
        if with_b1:
            im["b1t"] = np.ascontiguousarray(b1[e].reshape(MH, P).T)
        in_maps.append(im)

    res = None
    for attempt in range(3):
        try:
            res = bass_utils.run_bass_kernel_spmd(
                nc, in_maps, list(range(N_CORES)), trace=_trace
            )
            break
        except Exception:
            if attempt == 2:
                raise
    last_run_results = res

    out_flat = np.zeros((N, D), dtype=np.float32)
    for e in range(E):
        n_e = min(int(counts[e]), C)
        y_e = res.results[e]["yT"][:, :n_e].T   # [n_e, D] f32
        out_flat[idx[e][:n_e]] += gates[e][:n_e, None] * (y_e + b2[e])
        if counts[e] > C:  # host handles the overflow tokens
            hi = idx[e][C:]
            h = np.maximum(x_flat[hi] @ w1[e] + b1[e], 0.0)
            y = h @ w2[e] + b2[e]
            out_flat[hi] += gates[e][C:, None] * y
    return out_flat.reshape(B, S, D)


# revision 37
# speedup vs baseline: 1.2038x; 1.0052x over previous
"""MoE layer (top-2 of 8 experts, D=1024, H=2048) on 8 trn2 NeuronCores.

Strategy (expert-parallel, per the sharding hint):
  - Router (tiny: [16384,1024]@[1024,8]) runs on host; its output IS the
    sharding decision ("all-to-all tokens by expert assignment").
  - Core e receives the tokens routed to expert e (padded to a uniform
    capacity C, transposed to [D, C] bf16) plus expert e's weights, and
    computes yT = (relu(w1.T @ x + b1)).T-chain fully on-device:
        stage 1: hT[h, c] = relu(sum_d w1[d,h] * xT[d,c] + b1[h])
        stage 2: yT[d, c] = sum_h w2[h,d] * hT[h,c]
    bf16 matmuls, f32 PSUM accumulation, f32 output.
  - Host scatter-adds gate * (y + b2) into the output.
"""

import numpy as np
import ml_dtypes

import concourse.bacc as bacc
import concourse.mybir as mybir
import concourse.tile as tile
from concourse.tile_rust import add_dep_helper
from concourse import bass_utils

B, S, D, E, TOPK = 4, 4096, 1024, 8, 2
H = 2 * D
P = 128
KD = D // P    # 8 k-tiles over D
MH = H // P    # 16 h-tiles over H
ND = D // P    # 8 d-tiles over D
FD = 512       # moving free-dim per matmul / column block
N_CORES = 8

BF16 = mybir.dt.bfloat16
F32 = mybir.dt.float32

_cache = {}           # capacity C -> compiled Bacc
last_run_results = None  # BassKernelResults of the most recent device run


def _build(C, with_b1=True):
    """Build + compile the per-core FFN program for token capacity C.

    Weight dram layouts are tile-major (host pre-transposes):
      w1m[m, k, p, j] = w1[k*128+p, m*128+j]   (m-major: stage-1 weight
        column-tile m is a contiguous 32KB chunk per k -> the first
        matmul only needs w1m[0] + the first x block, so compute starts
        ~2us into the kernel instead of after the full 8MB weight load)
      w2d[d, m, p, j] = w2[m*128+p, d*128+j]   (d-major, same idea)
    """
    assert C % P == 0
    nc = bacc.Bacc("TRN2", target_bir_lowering=False, debug=False)
    xT = nc.dram_tensor("xT", [D, C], BF16, kind="ExternalInput").ap()
    w1m = nc.dram_tensor("w1m", [MH, P, KD, P], BF16, kind="ExternalInput").ap()
    w2d = nc.dram_tensor("w2d", [ND, P, MH, P], BF16, kind="ExternalInput").ap()
    b1t = (
        nc.dram_tensor("b1t", [P, MH], F32, kind="ExternalInput").ap()
        if with_b1 else None
    )
    yT = nc.dram_tensor("yT", [D, C], F32, kind="ExternalOutput").ap()

    blocks = []
    c0 = 0
    while c0 < C:
        fd = min(FD, C - c0)
        blocks.append((c0, fd))
        c0 += fd

    xT_r = xT.rearrange("(k p) c -> p k c", p=P)      # [P, KD, C]
    yT_r = yT.rearrange("(d p) c -> p d c", p=P)      # [P, ND, C]
    w1m_r = w1m.rearrange("m p k j -> p m (k j)")     # [P, MH, KD*P]
    w2d_r = w2d.rearrange("d p m j -> p d (m j)")     # [P, ND, MH*P]

    with tile.TileContext(nc) as tc:
        with (
            tc.tile_pool(name="wpool", bufs=1) as wpool,
            tc.tile_pool(name="xpool", bufs=3) as xpool,
            tc.tile_pool(name="hpool", bufs=3) as hpool,
            tc.tile_pool(name="ypool", bufs=2) as ypool,
            tc.tile_pool(name="ps1", bufs=4, space="PSUM") as ps1,
            tc.tile_pool(name="ps2", bufs=4, space="PSUM") as ps2,
        ):
            # First x block is the very first DMA on the sync ring: the
            # first matmul depends only on it and the first w1 chunk.
            xb0 = xpool.tile([P, KD, FD], BF16)
            c00, fd0 = blocks[0]
            nc.sync.dma_start(xb0[:, :, :fd0], xT_r[:, :, c00:c00 + fd0])
            if with_b1:
                b1_sb = wpool.tile([P, MH], F32)
                nc.gpsimd.dma_start(b1_sb[:], b1t[:, :])

            # w1 m=0..3 on the scalar HWDGE ring (lands early; only 4
            # triggers so the scalar engine is free for relu right after);
            # the bulk of the weights streams on the gpsimd SWDGE path as
            # a few large DMAs so no engine's trigger queue backs up.
            # A tiny SBUF->SBUF gpsimd DMA that reads xb0 gates the bulk
            # stream so it can't starve the first x block of HBM bandwidth.
            w1_sb = wpool.tile([P, MH, KD * P], BF16)
            for m in range(4):
                nc.scalar.dma_start(w1_sb[:, m, :], w1m_r[:, m, :])
            # DVE copy reading xb0 = completes only once xb0 is in SBUF;
            # every bulk weight DMA trigger gets a dep edge on it
            gate_sb = wpool.tile([P, 8], BF16)
            gate_copy = nc.vector.tensor_copy(gate_sb[:1, :8], xb0[:1, 0, :8])
            w2_sb = wpool.tile([P, ND, MH * P], BF16)
            bulk = []
            for m0 in range(4, MH, 4):
                bulk.append(nc.gpsimd.dma_start(
                    w1_sb[:, m0:m0 + 4, :], w1m_r[:, m0:m0 + 4, :]
                ))
            for d0 in range(0, ND, 4):
                bulk.append(nc.gpsimd.dma_start(
                    w2_sb[:, d0:d0 + 4, :], w2d_r[:, d0:d0 + 4, :]
                ))
            for w in bulk:
                add_dep_helper(
                    w.ins, gate_copy.ins,
                    reason="bulk weights wait for x chunk 0 to land",
                )

            xbs = {}

            def stage1(blki):
                c0, fd = blocks[blki]
                if blki == 0:
                    xb = xb0
                else:
                    xb = xpool.tile([P, KD, FD], BF16)
                    nc.sync.dma_start(xb[:, :, :fd], xT_r[:, :, c0:c0 + fd])
                hT = hpool.tile([P, MH, FD], BF16)
                xbs[blki] = (xb, hT)
                for m in range(MH):
                    ps = ps1.tile([P, FD], F32)
                    for k in range(KD):
                        nc.tensor.matmul(
                            ps[:, :fd],
                            w1_sb[:, m, k * P:(k + 1) * P],
                            xb[:, k, :fd],
                            start=(k == 0),
                            stop=(k == KD - 1),
                        )
                    nc.scalar.activation(
                        hT[:, m, :fd],
                        ps[:, :fd],
                        mybir.ActivationFunctionType.Relu,
                        bias=b1_sb[:, m:m + 1] if with_b1 else 0.0,
                    )

            def stage2(blki):
                c0, fd = blocks[blki]
                last = blki == len(blocks) - 1
                _, hT = xbs.pop(blki)
                yb = ypool.tile([P, ND, FD], F32)
                for d in range(ND):
                    ps = ps2.tile([P, FD], F32)
                    for m in range(MH):
                        nc.tensor.matmul(
                            ps[:, :fd],
                            w2_sb[:, d, m * P:(m + 1) * P],
                            hT[:, m, :fd],
                            start=(m == 0),
                            stop=(m == MH - 1),
                        )
                    nc.vector.tensor_copy(yb[:, d, :fd], ps[:, :fd])
                    if last:  # stream the tail out so the final DMA is tiny
                        nc.sync.dma_start(
                            yT_r[:, d, c0:c0 + fd], yb[:, d, :fd]
                        )
                if not last:
                    nc.sync.dma_start(yT_r[:, :, c0:c0 + fd], yb[:, :, :fd])

            # software-pipelined: stage 1 of block b+1 runs (on PE) between
            # stage 1 and stage 2 of block b, hiding the relu-eviction tail
            stage1(0)
            for b in range(len(blocks)):
                if b + 1 < len(blocks):
                    stage1(b + 1)
                stage2(b)

    nc.compile()
    return nc


def _route(x_flat, router_w, router_b):
    """Replicates the reference router in numpy float32."""
    logits = x_flat @ router_w + router_b            # [N, E]
    m = logits.max(axis=-1, keepdims=True)
    p = np.exp(logits - m, dtype=np.float32)
    p /= p.sum(axis=-1, keepdims=True)
    # top-k, ties -> lower index first (matches jax.lax.top_k)
    top_i = np.argsort(-p, axis=-1, kind="stable")[:, :TOPK]
    top_p = np.take_along_axis(p, top_i, axis=-1)
    top_p = top_p / top_p.sum(axis=-1, keepdims=True)
    return top_p, top_i


def kernel(x, router_w, router_b, w1, b1, w2, b2, _trace=False):
    global last_run_results
    x = np.asarray(x, dtype=np.float32)
    router_w = np.asarray(router_w, dtype=np.float32)
    router_b = np.asarray(router_b, dtype=np.float32)
    w1 = np.asarray(w1, dtype=np.float32)
    b1 = np.asarray(b1, dtype=np.float32)
    w2 = np.asarray(w2, dtype=np.float32)
    b2 = np.asarray(b2, dtype=np.float32)

    N = B * S
    x_flat = x.reshape(N, D)
    top_p, top_i = _route(x_flat, router_w, router_b)

    # Tokens per expert (the "all-to-all by expert assignment")
    idx = [np.nonzero((top_i == e).any(axis=-1))[0] for e in range(E)]
    gates = [
        (top_p[idx[e]] * (top_i[idx[e]] == e)).sum(axis=-1) for e in range(E)
    ]
    counts = np.array([len(i) for i in idx])
    # Device capacity: the perfect-balance point (N*K/8, multiple of 512).
    # The few overflow tokens of hotter-than-average experts are handled
    # on the host during the scatter-add (a data-parallel remainder).
    C = max(FD, int(-(-(N * TOPK // N_CORES) // FD) * FD))

    with_b1 = bool(np.any(b1))
    key = (C, with_b1)
    if key not in _cache:
        _cache[key] = _build(C, with_b1=with_b1)
    nc = _cache[key]

    in_maps = []
    for e in range(E):
        n_e = min(int(counts[e]), C)
        xTe = np.zeros((D, C), dtype=ml_dtypes.bfloat16)
        xTe[:, :n_e] = x_flat[idx[e][:n_e]].T
        w1m = np.ascontiguousarray(
            w1[e].reshape(KD, P, MH, P).transpose(2, 1, 0, 3)
        ).astype(ml_dtypes.bfloat16)
        w2d = np.ascontiguousarray(
            w2[e].reshape(MH, P, ND, P).transpose(2, 1, 0, 3)
        ).astype(ml_dtypes.bfloat16)
        im = {
            "xT": xTe,
            "w1m": w1m,
            "w2d": w2d,
        }
        if with_b1:
            im["b1t"] = np.ascontiguousarray(b1[e].reshape(MH, P).T)
        in_maps.append(im)

    res = None
    for attempt in range(3):
        try:
            res = bass_utils.run_bass_kernel_spmd(
                nc, in_maps, list(range(N_CORES)), trace=_trace
            )
            break
        except Exception:
            if attempt == 2:
                raise
    last_run_results = res

    out_flat = np.zeros((N, D), dtype=np.float32)
    for e in range(E):
        n_e = min(int(counts[e]), C)
        y_e = res.results[e]["yT"][:, :n_e].T   # [n_e, D] f32
        out_flat[idx[e][:n_e]] += gates[e][:n_e, None] * (y_e + b2[e])
        if counts[e] > C:  # host handles the overflow tokens
            hi = idx[e][C:]
            h = np.maximum(x_flat[hi] @ w1[e] + b1[e], 0.0)
            y = h @ w2[e] + b2[e]
            out_flat[hi] += gates[e][C:, None] * y
    return out_flat.reshape(B, S, D)
